# revision 28
# baseline (speedup 1.0000x reference)
"""LoRA multi-head attention on 8 TRN2 NeuronCores.

Sharding: data-parallel over batch (B=8 -> 1 batch element per core),
weights replicated, no collectives.

Host side: LoRA and the softmax scale are folded into the dense
weights (W'q = (Wq + Bq@Aq/16)/8, W'{k,v,o} = W + B@A/16) in fp32,
then transposed + cast bf16.  Mathematically identical to the
reference; removes the whole LoRA path from the device.

Device side per core, all bf16 with fp32 PSUM accumulation:
  qT/kT = W'T.T @ xT per 128-row dout tile; v natural [n, dout]
  stored per-head as [v_h | 1] so PV also yields softmax denoms.
  Attention runs in steps of two m-tiles: S^T for both heads via
  row-tiled (64x128) concurrent matmuls -> exp on ACT -> PV lagged
  one step (reads the previous step's probabilities, so it never
  waits on ACT) -> denominator row -> f16 K=1 ones-matmul broadcast
  -> fast reciprocal -> normalize straight into attnT.
  out = attnT.T @ WoT (+ bo via K=1 ones matmul only if bo != 0).

Scheduling: x/Wv are DMAed as column-half tiles ordered to feed the
kt-major v-projection waves (compute starts ~2us in, doubling as HAM
warmup); q/k projections for tile dt+1 are woven into tile dt's
attention; the first o-proj chains are woven into dt=7.
"""

import sys

if "/opt/trn_rl_repo" not in sys.path:
    sys.path.insert(0, "/opt/trn_rl_repo")

import numpy as np
import ml_dtypes

BF16 = ml_dtypes.bfloat16

N = 1024  # tokens
D = 1024  # model dim
H = 16    # heads
HD = 64   # head dim
P = 128   # partitions
F = 512   # psum free-dim tile
NCORES = 8
SCALING = 1.0 / 16.0  # lora alpha/rank
SCALE = HD ** -0.5

_CACHE = {}


def _build(with_bias):
    import concourse.bacc as bacc
    import concourse.mybir as mybir
    import concourse.tile as tile

    f32 = mybir.dt.float32
    f16 = mybir.dt.float16
    bf16 = mybir.dt.bfloat16
    Exp = mybir.ActivationFunctionType.Exp
    Copy = mybir.ActivationFunctionType.Copy

    nc = bacc.Bacc("TRN2", target_bir_lowering=False, debug=False)

    xT_e = nc.declare_dram_parameter("xT", [D, N], bf16, isOutput=False)
    wT_e = {
        nm: nc.declare_dram_parameter(nm, [D, D], bf16, isOutput=False)
        for nm in ("WvT", "WoT")
    }
    # Wq/Wk arrive in block-major layout [dt][kt][128,128] so only
    # dt=0's 0.5 MB gates the start of attention.
    wB_e = {
        nm: nc.declare_dram_parameter(nm, [64 * P, P], bf16, isOutput=False)
        for nm in ("WqB", "WkB")
    }
    bo_e = nc.declare_dram_parameter("BoT", [1, D], bf16, isOutput=False)
    out_e = nc.declare_dram_parameter("out", [N, D], bf16, isOutput=True)

    with tile.TileContext(nc) as tc:
        with (
            tc.tile_pool(name="wpool", bufs=1) as wpool,
            tc.tile_pool(name="stage", bufs=2) as stage,
            tc.tile_pool(name="ps", bufs=1, space="PSUM") as ps,
        ):
            qs = [nc.sync, nc.scalar, nc.gpsimd]
            qi = 0

            def dma_tile(dst, src):
                nonlocal qi
                qs[qi % len(qs)].dma_start(out=dst[:], in_=src)
                qi += 1

            # x and Wv live as column-half tiles [P, F]; DMA order is
            # chosen so each v-wave's inputs land just before it runs.
            xh = [[wpool.tile([P, F], bf16, tag=f"x_{t}_{h}",
                              name=f"x_{t}_{h}") for h in range(2)]
                  for t in range(8)]
            vh = [[wpool.tile([P, F], bf16, tag=f"Wv_{t}_{h}",
                              name=f"Wv_{t}_{h}") for h in range(2)]
                  for t in range(8)]
            T = {}
            for nm in ("Wo",):
                T[nm] = [wpool.tile([P, D], bf16, tag=f"T_{nm}_{t}",
                                    name=f"T_{nm}_{t}") for t in range(8)]
            WB = {nm: [wpool.tile([P, D], bf16, tag=f"{nm}_{dt}",
                                  name=f"{nm}_{dt}") for dt in range(8)]
                  for nm in ("Wq", "Wk")}
            for t in range(8):
                dma_tile(xh[t][0], xT_e[t * P:(t + 1) * P, 0:F])
                dma_tile(vh[t][0], wT_e["WvT"][t * P:(t + 1) * P, 0:F])
            for t in range(8):
                dma_tile(vh[t][1], wT_e["WvT"][t * P:(t + 1) * P, F:2 * F])
            for t in range(8):
                dma_tile(xh[t][1], xT_e[t * P:(t + 1) * P, F:2 * F])
            for dt in range(8):
                for nm, e in (("Wq", wB_e["WqB"]), ("Wk", wB_e["WkB"])):
                    for kt in range(8):
                        r0 = (dt * 8 + kt) * P
                        dma_tile(WB[nm][dt][:, kt * P:(kt + 1) * P],
                                 e[r0:r0 + P, :])
            for t in range(8):
                dma_tile(T["Wo"][t], wT_e["WoT"][t * P:(t + 1) * P, :])
            bot = wpool.tile([1, D], bf16, tag="bot")
            dma_tile(bot, bo_e[:, :])

            def xcol(t, c0, c1):  # x[t] columns [c0, c1) across halves
                h = c0 // F
                return xh[t][h][:, c0 - h * F:c1 - h * F]

            onesh = wpool.tile([1, HD], f16, tag="onesh")
            nc.vector.memset(onesh[:], 1.0)
            onesb = wpool.tile([1, P], bf16, tag="onesb")
            nc.vector.memset(onesb[:], 1.0)

            # ---- tiny HAM kick, gated on the first x half-tile ----
            wps = ps.tile([P, F], f32, tag="projpsum", bufs=2)
            for _ in range(6):
                nc.tensor.matmul(wps[:, 0:256], xh[0][0][:, 0:P],
                                 xh[0][0][:, 0:256], start=True, stop=True)

            # ---- v natural, per-head layout [v_h | 1]; kt-major waves
            # gated on the half-tile DMA arrivals. ----
            VW = H * (HD + 1)  # 1040
            v_sb = [wpool.tile([P, VW], bf16, tag=f"v_{t}",
                               name=f"v_{t}") for t in range(8)]
            # ---- q/k projection generator (dense only; lora folded) ----
            qks = {}

            def proj_gen(dt):
                qk = {}
                for nm, wnm in (("q", "Wq"), ("k", "Wk")):
                    dst = wpool.tile([P, D], bf16, tag=f"{nm}T",
                                     bufs=3, name=f"{nm}T_{dt}")
                    qk[nm] = dst
                    for nh in range(2):
                        ns = slice(nh * F, (nh + 1) * F)
                        pq = ps.tile([P, F], f32, tag="projpsum", bufs=2)
                        for kt in range(8):
                            nc.tensor.matmul(
                                pq[:], WB[wnm][dt][:, kt * P:(kt + 1) * P],
                                xh[kt][nh][:],
                                start=(kt == 0), stop=(kt == 7))
                            yield
                        nc.vector.tensor_copy(dst[:, ns], pq[:])
                        yield
                qks[dt] = qk

            # ---- v waves, kt-major, gated on half-tile DMA arrivals;
            # proj(0) woven into waves 2-3 (its blocks land by then) ----
            g0 = proj_gen(0)
            waves = [[(nt, 0) for nt in range(4)],
                     [(nt, 1) for nt in range(4)],
                     [(nt, 0) for nt in range(4, 8)],
                     [(nt, 1) for nt in range(4, 8)]]
            for wv, chains in enumerate(waves):
                pv = {}
                for ci, (nt, dh) in enumerate(chains):
                    if wv == 3 and ci >= 2:
                        # last wave's tail chains use pvpsum so a spair
                        # tile is already free when attention starts
                        pvt2 = ps.tile([P, F], f32, tag="pvpsum", bufs=2,
                                       name=f"pvp_{wv}_{ci}")
                        pv[(nt, dh)] = pvt2[:]
                        continue
                    if ci % 2 == 0:
                        pvt = ps.tile([P, 2 * F], f32, tag="spair", bufs=2,
                                      name=f"pvt_{wv}_{ci}")
                    pv[(nt, dh)] = pvt[:, (ci % 2) * F:(ci % 2 + 1) * F]
                for kt in range(8):
                    for (nt, dh) in chains:
                        nc.tensor.matmul(
                            pv[(nt, dh)], xcol(kt, nt * P, (nt + 1) * P),
                            vh[kt][dh][:],
                            start=(kt == 0), stop=(kt == 7))
                    if wv >= 2:
                        for _ in range(3):
                            next(g0, None)
                for (nt, dh) in chains:
                    vr = v_sb[nt][:].rearrange("p (h c) -> p h c", c=HD + 1)
                    pvr = pv[(nt, dh)].rearrange("p (h c) -> p h c", c=HD)
                    nc.vector.tensor_copy(
                        vr[:, dh * 8:(dh + 1) * 8, 0:HD], pvr[:])
                    if wv >= 2:
                        for _ in range(2):
                            next(g0, None)
                    if dh == 1:
                        nc.vector.memset(vr[:, :, HD:HD + 1], 1.0)
            for _ in g0:
                pass

            # ---- output projection chain pieces ----
            attnT = [wpool.tile([P, D], bf16, tag=f"attnT_{t}",
                                name=f"attnT_{t}") for t in range(8)]

            def oproj_head(nt, dh, upto):
                pf = ps.tile([P, F], f32, tag="projpsum", bufs=2,
                             name=f"pf_{nt}_{dh}")
                for kt in range(upto):
                    nc.tensor.matmul(pf[:],
                                     attnT[kt][:, nt * P:(nt + 1) * P],
                                     T["Wo"][kt][:, dh * F:(dh + 1) * F],
                                     start=(kt == 0),
                                     stop=(kt == 7 and not with_bias))
                    yield pf

            def oproj_tail(pf, nt, dh, upto, on_act):
                ds = slice(dh * F, (dh + 1) * F)
                for kt in range(upto, 8):
                    nc.tensor.matmul(pf[:],
                                     attnT[kt][:, nt * P:(nt + 1) * P],
                                     T["Wo"][kt][:, ds],
                                     start=False,
                                     stop=(kt == 7 and not with_bias))
                if with_bias:
                    nc.tensor.matmul(pf[:], onesb[0:1, :], bot[0:1, ds],
                                     start=False, stop=True)
                osb = stage.tile([P, F], bf16, tag="osb", bufs=3)
                if on_act:
                    nc.scalar.activation(osb[:], pf[:], Copy)
                else:
                    nc.vector.tensor_copy(osb[:], pf[:])
                oq = [nc.sync, nc.gpsimd, nc.scalar][(nt * 2 + dh) % 3]
                oq.dma_start(out=out_e[nt * P:(nt + 1) * P, ds],
                             in_=osb[:])

            def oproj_chain(nt, dh, on_act):
                pf = None
                for pf in oproj_head(nt, dh, 8):
                    yield
                oproj_tail(pf, nt, dh, 8, on_act)
                yield

            # ---- attention per dout-tile, two m-tiles per step with
            # PV lagged one step so it never waits on ACT ----
            woven = [(0, 0), (0, 1)]
            for dt in range(8):
                if dt < 7:
                    g = proj_gen(dt + 1)
                else:
                    def _dt7_gen():
                        for nt, dh in woven:
                            pfh = None
                            for pfh in oproj_head(nt, dh, 7):
                                yield
                            _pf_held.append((pfh, nt, dh))
                    _pf_held = []
                    g = _dt7_gen()
                h0 = 2 * dt
                qt = qks[dt]["q"]
                ktt = qks[dt]["k"]
                for nh in range(2):
                    ns = slice(nh * F, (nh + 1) * F)
                    po = {}
                    for h in (h0, h0 + 1):
                        po[h] = ps.tile([HD + 1, F], f32, tag="pvpsum",
                                        bufs=2, name=f"po_{h}_{nh}")
                    pte = {}

                    def qk_exp(mt):
                        spair = ps.tile([P, 2 * F], f32, tag="spair",
                                        bufs=2)
                        for hi, h in enumerate((h0, h0 + 1)):
                            ro = (h % 2) * HD
                            m0 = mt * P
                            nc.tensor.matmul(
                                spair[:, hi * F:(hi + 1) * F],
                                ktt[ro:ro + HD, m0:m0 + P],
                                qt[ro:ro + HD, ns], start=True, stop=True)
                        pte[mt] = stage.tile([P, 2 * F], bf16, tag="pt",
                                             bufs=4, name=f"pte_{mt}")
                        nc.scalar.activation(pte[mt][:], spair[:], Exp)

                    def pv(mt):
                        for hi, h in enumerate((h0, h0 + 1)):
                            nc.tensor.matmul(
                                po[h][:],
                                v_sb[mt][:, h * (HD + 1):(h + 1) * (HD + 1)],
                                pte[mt][:, hi * F:(hi + 1) * F],
                                start=(mt == 0), stop=(mt == 7))

                    for step in range(4):
                        qk_exp(2 * step)
                        qk_exp(2 * step + 1)
                        if step > 0:
                            pv(2 * step - 2)
                            pv(2 * step - 1)
                        for _ in range(3):
                            next(g, None)
                    pv(6)
                    pv(7)
                    # finalize both heads: DVE prep, batched broadcasts,
                    # then reciprocals + normalize into attnT
                    oah = {}
                    dn = {}
                    pbp = {}
                    for h in (h0, h0 + 1):
                        oah[h] = stage.tile([HD + 1, F], f32, tag="oah",
                                            bufs=3, name=f"oah_{h}")
                        nc.vector.tensor_copy(oah[h][:], po[h][:])
                        dn[h] = stage.tile([1, F], f16, tag="dn", bufs=3,
                                           name=f"dn_{h}")
                        nc.vector.tensor_copy(dn[h][:], oah[h][HD:HD + 1, :])
                    for h in (h0, h0 + 1):
                        pbp[h] = ps.tile([HD, F], f32, tag="pvpsum",
                                         bufs=2, name=f"pb_{h}")
                        nc.tensor.matmul(pbp[h][:], onesh[0:1, :], dn[h][:],
                                         start=True, stop=True)
                    for _ in range(2):
                        next(g, None)
                    for h in (h0, h0 + 1):
                        ro = (h % 2) * HD
                        pbs = stage.tile([HD, F], f32, tag="pbs", bufs=3,
                                         name=f"pbs_{h}")
                        nc.vector.reciprocal_approx_fast(pbs[:], pbp[h][:])
                        nc.vector.tensor_mul(attnT[dt][ro:ro + HD, ns],
                                             oah[h][0:HD, :], pbs[:])
                for _ in g:
                    pass

            # ---- finish woven o-proj chains, then the rest ----
            for i, (pfh, nt, dh) in enumerate(_pf_held):
                oproj_tail(pfh, nt, dh, 7, on_act=(i % 2 == 1))
            rest = [(nt, dh) for nt in range(8) for dh in range(2)
                    if (nt, dh) not in woven]
            for i, (nt, dh) in enumerate(rest):
                for _ in oproj_chain(nt, dh, on_act=(i % 2 == 1)):
                    pass
    nc.compile()
    return nc


def _get_nc(with_bias=False):
    key = ("nc", with_bias)
    if key not in _CACHE:
        _CACHE[key] = _build(with_bias)
    return _CACHE[key]


def _prep_shared(inputs):
    def fold(w, a, b, scl=1.0):
        w = np.asarray(w, np.float32)
        a = np.asarray(a, np.float32)
        b = np.asarray(b, np.float32)
        eff = (w + (b @ a) * SCALING) * scl
        return np.ascontiguousarray(eff.T.astype(BF16))

    def blocks(wt):  # [D, D] -> [dt][kt][128,128] block-major
        b = wt.reshape(8, P, 8, P).transpose(2, 0, 1, 3)
        return np.ascontiguousarray(b.reshape(64 * P, P))

    shared = {
        "WqB": blocks(fold(inputs["Wq"], inputs["Aq"], inputs["Bq"], SCALE)),
        "WkB": blocks(fold(inputs["Wk"], inputs["Ak"], inputs["Bk"])),
        "WvT": fold(inputs["Wv"], inputs["Av"], inputs["Bv"]),
        "WoT": fold(inputs["Wo"], inputs["Ao"], inputs["Bo"]),
        "BoT": np.ascontiguousarray(
            np.asarray(inputs["bo"], np.float32).reshape(1, D).astype(BF16)),
    }
    return shared


def kernel(**inputs):
    from concourse import bass_utils

    with_bias = bool(np.any(np.asarray(inputs["bo"], np.float32)))
    nc = _get_nc(with_bias)
    shared = _prep_shared(inputs)
    x = np.asarray(inputs["x"], np.float32)
    in_maps = []
    for i in range(NCORES):
        m = dict(shared)
        m["xT"] = np.ascontiguousarray(x[i].T.astype(BF16))
        in_maps.append(m)
    res = bass_utils.run_bass_kernel_spmd(nc, in_maps,
                                          core_ids=list(range(NCORES)))
    return np.stack([np.asarray(res.results[i]["out"]).astype(np.float32)
                     for i in range(NCORES)], axis=0)


# revision 29
# speedup vs baseline: 1.0292x; 1.0292x over previous
"""LoRA multi-head attention on 8 TRN2 NeuronCores.

Sharding: data-parallel over batch (B=8 -> 1 batch element per core),
weights replicated, no collectives.

Host side: LoRA and the softmax scale are folded into the dense
weights (W'q = (Wq + Bq@Aq/16)/8, W'{k,v,o} = W + B@A/16) in fp32,
then transposed + cast bf16.  Mathematically identical to the
reference; removes the whole LoRA path from the device.

Device side per core, all bf16 with fp32 PSUM accumulation:
  qT/kT = W'T.T @ xT per 128-row dout tile; v natural [n, dout]
  stored per-head as [v_h | 1] so PV also yields softmax denoms.
  Attention runs in steps of two m-tiles: S^T for both heads via
  row-tiled (64x128) concurrent matmuls -> exp on ACT -> PV lagged
  one step (reads the previous step's probabilities, so it never
  waits on ACT) -> denominator row -> f16 K=1 ones-matmul broadcast
  -> fast reciprocal -> normalize straight into attnT.
  out = attnT.T @ WoT (+ bo via K=1 ones matmul only if bo != 0).

Scheduling: x/Wv are DMAed as column-half tiles ordered to feed the
kt-major v-projection waves (compute starts ~2us in, doubling as HAM
warmup); q/k projections for tile dt+1 are woven into tile dt's
attention; the first o-proj chains are woven into dt=7.
"""

import sys

if "/opt/trn_rl_repo" not in sys.path:
    sys.path.insert(0, "/opt/trn_rl_repo")

import numpy as np
import ml_dtypes

BF16 = ml_dtypes.bfloat16

N = 1024  # tokens
D = 1024  # model dim
H = 16    # heads
HD = 64   # head dim
P = 128   # partitions
F = 512   # psum free-dim tile
NCORES = 8
SCALING = 1.0 / 16.0  # lora alpha/rank
SCALE = HD ** -0.5

_CACHE = {}


def _build(with_bias):
    import concourse.bacc as bacc
    import concourse.mybir as mybir
    import concourse.tile as tile

    f32 = mybir.dt.float32
    f16 = mybir.dt.float16
    bf16 = mybir.dt.bfloat16
    Exp = mybir.ActivationFunctionType.Exp
    Copy = mybir.ActivationFunctionType.Copy

    nc = bacc.Bacc("TRN2", target_bir_lowering=False, debug=False)

    xT_e = nc.declare_dram_parameter("xT", [D, N], bf16, isOutput=False)
    wT_e = {
        nm: nc.declare_dram_parameter(nm, [D, D], bf16, isOutput=False)
        for nm in ("WvT", "WoT")
    }
    # Wq/Wk arrive in block-major layout [dt][kt][128,128] so only
    # dt=0's 0.5 MB gates the start of attention.
    wB_e = {
        nm: nc.declare_dram_parameter(nm, [64 * P, P], bf16, isOutput=False)
        for nm in ("WqB", "WkB")
    }
    bo_e = nc.declare_dram_parameter("BoT", [1, D], bf16, isOutput=False)
    out_e = nc.declare_dram_parameter("out", [N, D], bf16, isOutput=True)

    with tile.TileContext(nc) as tc:
        with (
            tc.tile_pool(name="wpool", bufs=1) as wpool,
            tc.tile_pool(name="stage", bufs=2) as stage,
            tc.tile_pool(name="ps", bufs=1, space="PSUM") as ps,
        ):
            qs = [nc.sync, nc.scalar, nc.gpsimd]
            qi = 0

            def dma_tile(dst, src):
                nonlocal qi
                qs[qi % len(qs)].dma_start(out=dst[:], in_=src)
                qi += 1

            # x and Wv live as column-half tiles [P, F]; DMA order is
            # chosen so each v-wave's inputs land just before it runs.
            xh = [[wpool.tile([P, F], bf16, tag=f"x_{t}_{h}",
                              name=f"x_{t}_{h}") for h in range(2)]
                  for t in range(8)]
            vh = [[wpool.tile([P, F], bf16, tag=f"Wv_{t}_{h}",
                              name=f"Wv_{t}_{h}") for h in range(2)]
                  for t in range(8)]
            T = {}
            for nm in ("Wo",):
                T[nm] = [wpool.tile([P, D], bf16, tag=f"T_{nm}_{t}",
                                    name=f"T_{nm}_{t}") for t in range(8)]
            WB = {nm: [[wpool.tile([P, P], bf16, tag=f"{nm}_{dt}_{kt}",
                                   name=f"{nm}_{dt}_{kt}")
                        for kt in range(8)] for dt in range(8)]
                  for nm in ("Wq", "Wk")}
            for t in range(8):
                dma_tile(xh[t][0], xT_e[t * P:(t + 1) * P, 0:F])
                dma_tile(vh[t][0], wT_e["WvT"][t * P:(t + 1) * P, 0:F])
            for t in range(8):
                dma_tile(vh[t][1], wT_e["WvT"][t * P:(t + 1) * P, F:2 * F])
            for t in range(8):
                dma_tile(xh[t][1], xT_e[t * P:(t + 1) * P, F:2 * F])
            for dt in range(8):
                for nm, e in (("Wq", wB_e["WqB"]), ("Wk", wB_e["WkB"])):
                    for kt in range(8):
                        r0 = (dt * 8 + kt) * P
                        dma_tile(WB[nm][dt][kt], e[r0:r0 + P, :])
            for t in range(8):
                dma_tile(T["Wo"][t], wT_e["WoT"][t * P:(t + 1) * P, :])
            bot = wpool.tile([1, D], bf16, tag="bot")
            dma_tile(bot, bo_e[:, :])

            def xcol(t, c0, c1):  # x[t] columns [c0, c1) across halves
                h = c0 // F
                return xh[t][h][:, c0 - h * F:c1 - h * F]

            onesh = wpool.tile([1, HD], f16, tag="onesh")
            nc.vector.memset(onesh[:], 1.0)
            onesb = wpool.tile([1, P], bf16, tag="onesb")
            nc.vector.memset(onesb[:], 1.0)

            # ---- tiny HAM kick, gated on the first x half-tile ----
            wps = ps.tile([P, F], f32, tag="projpsum", bufs=2)
            for _ in range(6):
                nc.tensor.matmul(wps[:, 0:256], xh[0][0][:, 0:P],
                                 xh[0][0][:, 0:256], start=True, stop=True)

            # ---- v natural, per-head layout [v_h | 1]; kt-major waves
            # gated on the half-tile DMA arrivals. ----
            VW = H * (HD + 1)  # 1040
            v_sb = [wpool.tile([P, VW], bf16, tag=f"v_{t}",
                               name=f"v_{t}") for t in range(8)]
            # ---- q/k projection generator (dense only; lora folded) ----
            qks = {}

            def proj_gen(dt):
                qk = {}
                for nm, wnm in (("q", "Wq"), ("k", "Wk")):
                    dst = wpool.tile([P, D], bf16, tag=f"{nm}T",
                                     bufs=3, name=f"{nm}T_{dt}")
                    qk[nm] = dst
                    for nh in range(2):
                        ns = slice(nh * F, (nh + 1) * F)
                        pq = ps.tile([P, F], f32, tag="projpsum", bufs=2)
                        for kt in range(8):
                            nc.tensor.matmul(
                                pq[:], WB[wnm][dt][kt][:],
                                xh[kt][nh][:],
                                start=(kt == 0), stop=(kt == 7))
                            yield
                        nc.vector.tensor_copy(dst[:, ns], pq[:])
                        yield
                qks[dt] = qk

            # ---- v waves, kt-major, gated on half-tile DMA arrivals;
            # proj(0) woven into waves 2-3 (its blocks land by then) ----
            g0 = proj_gen(0)
            waves = [[(nt, 0) for nt in range(4)],
                     [(nt, 1) for nt in range(4)],
                     [(nt, 0) for nt in range(4, 8)],
                     [(nt, 1) for nt in range(4, 8)]]
            for wv, chains in enumerate(waves):
                pv = {}
                for ci, (nt, dh) in enumerate(chains):
                    if wv == 3 and ci >= 2:
                        # last wave's tail chains use pvpsum so a spair
                        # tile is already free when attention starts
                        pvt2 = ps.tile([P, F], f32, tag="pvpsum", bufs=2,
                                       name=f"pvp_{wv}_{ci}")
                        pv[(nt, dh)] = pvt2[:]
                        continue
                    if ci % 2 == 0:
                        pvt = ps.tile([P, 2 * F], f32, tag="spair", bufs=2,
                                      name=f"pvt_{wv}_{ci}")
                    pv[(nt, dh)] = pvt[:, (ci % 2) * F:(ci % 2 + 1) * F]
                for kt in range(8):
                    for (nt, dh) in chains:
                        nc.tensor.matmul(
                            pv[(nt, dh)], xcol(kt, nt * P, (nt + 1) * P),
                            vh[kt][dh][:],
                            start=(kt == 0), stop=(kt == 7))
                    if wv >= 2:
                        for _ in range(3):
                            next(g0, None)
                for (nt, dh) in chains:
                    vr = v_sb[nt][:].rearrange("p (h c) -> p h c", c=HD + 1)
                    pvr = pv[(nt, dh)].rearrange("p (h c) -> p h c", c=HD)
                    nc.vector.tensor_copy(
                        vr[:, dh * 8:(dh + 1) * 8, 0:HD], pvr[:])
                    if wv >= 2:
                        for _ in range(2):
                            next(g0, None)
                    if dh == 1:
                        nc.vector.memset(vr[:, :, HD:HD + 1], 1.0)
            for _ in g0:
                pass

            # ---- output projection chain pieces ----
            attnT = [wpool.tile([P, D], bf16, tag=f"attnT_{t}",
                                name=f"attnT_{t}") for t in range(8)]

            def oproj_head(nt, dh, upto):
                pf = ps.tile([P, F], f32, tag="projpsum", bufs=2,
                             name=f"pf_{nt}_{dh}")
                for kt in range(upto):
                    nc.tensor.matmul(pf[:],
                                     attnT[kt][:, nt * P:(nt + 1) * P],
                                     T["Wo"][kt][:, dh * F:(dh + 1) * F],
                                     start=(kt == 0),
                                     stop=(kt == 7 and not with_bias))
                    yield pf

            def oproj_tail(pf, nt, dh, upto, on_act):
                ds = slice(dh * F, (dh + 1) * F)
                for kt in range(upto, 8):
                    nc.tensor.matmul(pf[:],
                                     attnT[kt][:, nt * P:(nt + 1) * P],
                                     T["Wo"][kt][:, ds],
                                     start=False,
                                     stop=(kt == 7 and not with_bias))
                if with_bias:
                    nc.tensor.matmul(pf[:], onesb[0:1, :], bot[0:1, ds],
                                     start=False, stop=True)
                osb = stage.tile([P, F], bf16, tag="osb", bufs=3)
                if on_act:
                    nc.scalar.activation(osb[:], pf[:], Copy)
                else:
                    nc.vector.tensor_copy(osb[:], pf[:])
                oq = [nc.sync, nc.gpsimd, nc.scalar][(nt * 2 + dh) % 3]
                oq.dma_start(out=out_e[nt * P:(nt + 1) * P, ds],
                             in_=osb[:])

            def oproj_chain(nt, dh, on_act):
                pf = None
                for pf in oproj_head(nt, dh, 8):
                    yield
                oproj_tail(pf, nt, dh, 8, on_act)
                yield

            # ---- attention per dout-tile, two m-tiles per step with
            # PV lagged one step so it never waits on ACT ----
            woven = [(0, 0), (0, 1)]
            for dt in range(8):
                if dt < 7:
                    g = proj_gen(dt + 1)
                else:
                    def _dt7_gen():
                        for nt, dh in woven:
                            pfh = None
                            for pfh in oproj_head(nt, dh, 7):
                                yield
                            _pf_held.append((pfh, nt, dh))
                    _pf_held = []
                    g = _dt7_gen()
                h0 = 2 * dt
                qt = qks[dt]["q"]
                ktt = qks[dt]["k"]
                for nh in range(2):
                    ns = slice(nh * F, (nh + 1) * F)
                    po = {}
                    for h in (h0, h0 + 1):
                        po[h] = ps.tile([HD + 1, F], f32, tag="pvpsum",
                                        bufs=2, name=f"po_{h}_{nh}")
                    pte = {}

                    def qk_exp(mt):
                        spair = ps.tile([P, 2 * F], f32, tag="spair",
                                        bufs=2)
                        for hi, h in enumerate((h0, h0 + 1)):
                            ro = (h % 2) * HD
                            m0 = mt * P
                            nc.tensor.matmul(
                                spair[:, hi * F:(hi + 1) * F],
                                ktt[ro:ro + HD, m0:m0 + P],
                                qt[ro:ro + HD, ns], start=True, stop=True)
                        pte[mt] = stage.tile([P, 2 * F], bf16, tag="pt",
                                             bufs=4, name=f"pte_{mt}")
                        nc.scalar.activation(pte[mt][:], spair[:], Exp)

                    def pv(mt):
                        for hi, h in enumerate((h0, h0 + 1)):
                            nc.tensor.matmul(
                                po[h][:],
                                v_sb[mt][:, h * (HD + 1):(h + 1) * (HD + 1)],
                                pte[mt][:, hi * F:(hi + 1) * F],
                                start=(mt == 0), stop=(mt == 7))

                    for step in range(4):
                        qk_exp(2 * step)
                        qk_exp(2 * step + 1)
                        if step > 0:
                            pv(2 * step - 2)
                            pv(2 * step - 1)
                        for _ in range(3):
                            next(g, None)
                    pv(6)
                    pv(7)
                    # finalize both heads: DVE prep, batched broadcasts,
                    # then reciprocals + normalize into attnT
                    oah = {}
                    dn = {}
                    pbp = {}
                    for h in (h0, h0 + 1):
                        oah[h] = stage.tile([HD + 1, F], f32, tag="oah",
                                            bufs=3, name=f"oah_{h}")
                        nc.vector.tensor_copy(oah[h][:], po[h][:])
                        dn[h] = stage.tile([1, F], f16, tag="dn", bufs=3,
                                           name=f"dn_{h}")
                        nc.vector.tensor_copy(dn[h][:], oah[h][HD:HD + 1, :])
                    for h in (h0, h0 + 1):
                        pbp[h] = ps.tile([HD, F], f32, tag="pvpsum",
                                         bufs=2, name=f"pb_{h}")
                        nc.tensor.matmul(pbp[h][:], onesh[0:1, :], dn[h][:],
                                         start=True, stop=True)
                    for _ in range(2):
                        next(g, None)
                    for h in (h0, h0 + 1):
                        ro = (h % 2) * HD
                        pbs = stage.tile([HD, F], f32, tag="pbs", bufs=3,
                                         name=f"pbs_{h}")
                        nc.vector.reciprocal_approx_fast(pbs[:], pbp[h][:])
                        nc.vector.tensor_mul(attnT[dt][ro:ro + HD, ns],
                                             oah[h][0:HD, :], pbs[:])
                for _ in g:
                    pass

            # ---- finish woven o-proj chains, then the rest ----
            for i, (pfh, nt, dh) in enumerate(_pf_held):
                oproj_tail(pfh, nt, dh, 7, on_act=(i % 2 == 1))
            rest = [(nt, dh) for nt in range(8) for dh in range(2)
                    if (nt, dh) not in woven]
            for i, (nt, dh) in enumerate(rest):
                for _ in oproj_chain(nt, dh, on_act=(i % 2 == 1)):
                    pass
    nc.compile()
    return nc


def _get_nc(with_bias=False):
    key = ("nc", with_bias)
    if key not in _CACHE:
        _CACHE[key] = _build(with_bias)
    return _CACHE[key]


def _prep_shared(inputs):
    def fold(w, a, b, scl=1.0):
        w = np.asarray(w, np.float32)
        a = np.asarray(a, np.float32)
        b = np.asarray(b, np.float32)
        eff = (w + (b @ a) * SCALING) * scl
        return np.ascontiguousarray(eff.T.astype(BF16))

    def blocks(wt):  # [D, D] -> [dt][kt][128,128] block-major
        b = wt.reshape(8, P, 8, P).transpose(2, 0, 1, 3)
        return np.ascontiguousarray(b.reshape(64 * P, P))

    shared = {
        "WqB": blocks(fold(inputs["Wq"], inputs["Aq"], inputs["Bq"], SCALE)),
        "WkB": blocks(fold(inputs["Wk"], inputs["Ak"], inputs["Bk"])),
        "WvT": fold(inputs["Wv"], inputs["Av"], inputs["Bv"]),
        "WoT": fold(inputs["Wo"], inputs["Ao"], inputs["Bo"]),
        "BoT": np.ascontiguousarray(
            np.asarray(inputs["bo"], np.float32).reshape(1, D).astype(BF16)),
    }
    return shared


def kernel(**inputs):
    from concourse import bass_utils

    with_bias = bool(np.any(np.asarray(inputs["bo"], np.float32)))
    nc = _get_nc(with_bias)
    shared = _prep_shared(inputs)
    x = np.asarray(inputs["x"], np.float32)
    in_maps = []
    for i in range(NCORES):
        m = dict(shared)
        m["xT"] = np.ascontiguousarray(x[i].T.astype(BF16))
        in_maps.append(m)
    res = bass_utils.run_bass_kernel_spmd(nc, in_maps,
                                          core_ids=list(range(NCORES)))
    return np.stack([np.asarray(res.results[i]["out"]).astype(np.float32)
                     for i in range(NCORES)], axis=0)
